# revision 37
# baseline (speedup 1.0000x reference)
"""Multi-head causal attention (RoPE) forward on 8 Trainium2 NeuronCores.

Sharding: tensor-parallel over heads -- 8 cores x 2 heads, each core handling
both batch elements (the flattened (B*T) = 4096 "time" axis).

v2: all matmul operands fp16 (PSUM accumulation stays fp32; measured end-to-end
rel err ~1e-3, gate is 2e-2), host-retiled contiguous DMA layouts, TC1=512,
phase-2 bias-adds and broadcasts on the idle GpSimd engine, reciprocal taken
after the partition-broadcast (128-partition op, not 1-partition), per-head
AllToAll in fp16 with the output projection split into head-parity halves so
the last a2a overlaps the first half's matmuls.

Per core:
  phase 1: qT/kT [d, B*T] and v [B*T, d] projections from host-pre-tiled
           xT, RoPE applied via a +-1 pair-swap permutation matmul on PE plus
           elementwise combine with host-precomputed interleaved cos/sin.
  phase 2: per (head, batch), scores^T [j, i] = kT^T @ qT, exp on ScalarE (no
           max pass -- the score distribution is bounded and softmax is
           shift-invariant), mask as additive bias on partially-masked tiles
           only, fully-masked tiles skipped; out^T [d, i] and the softmax
           denominators (ones-matmul) accumulate on PE; normalized on evac.
           Each head's attention output leaves via its own 8-rank AllToAll
           (head-split -> t-split) so comm overlaps the next head's compute.
  phase 3: y[t-slice, :] = outT_full^T @ wo, accumulated in two head-parity
           groups so only the first group waits on the final AllToAll.
Host assembles the 8 t-slices into the full (B, T, C) output.
"""

import os
import sys

import numpy as np

for _p in ("/opt/trn_rl_repo", "/root/.axon_site/_ro/trn_rl_repo"):
    if os.path.isdir(_p) and _p not in sys.path:
        sys.path.append(_p)

import concourse.bacc as bacc
import concourse.tile as tile
from concourse import mybir
from concourse.bass_utils import run_bass_kernel_spmd

B, T, C = 2, 2048, 2048
N_HEADS, D = 16, 128
THETA = 10000.0
N_CORES = 8
HPC = N_HEADS // N_CORES     # heads per core
BT = B * T                   # flattened time axis
TSL = BT // N_CORES          # per-core output slice after the all-to-all
KT = C // 128                # contraction chunks
TC1 = 512                    # phase-1 t-chunk (moving free dim)
NTC1 = BT // TC1
TC2 = 512                    # phase-2/3 chunk
CI = T // TC2                # i-chunks per (head, batch)
JT = T // 128                # j-tiles per (head, batch)
SCALE = 1.0 / np.sqrt(D)
MASKED_BIAS = -1.0e6         # pre-scale units; exp(SCALE*(s+bias)) == 0

F16 = mybir.dt.float16
F32 = mybir.dt.float32


def _mask_plan(mask2d):
    """Per (ci, jt): None=skip (all masked), (-1, 0)=free (none masked),
    (idx, lo) = partial tile whose columns [lo, TC2) have at least one valid
    row; idx is a 0/1 multiplier tile applied to exp(scores) on that column
    range. scoresT tile (jt, ci) holds mask2d[i, j] transposed:
    m01[j_loc, i_loc] <- mask2d[TC2*ci+i, 128*jt+j].
    """
    uniq = {}
    tiles = []
    plan = []
    for ci in range(CI):
        row = []
        for jt in range(JT):
            blk = mask2d[TC2 * ci:TC2 * (ci + 1), 128 * jt:128 * (jt + 1)]
            if blk.all():
                row.append((-1, 0))
            elif not blk.any():
                row.append(None)
            else:
                m01 = blk.T.astype(np.float16)   # [128 j, TC2 i]
                lo = int(np.argmax(m01.any(axis=0)))
                key = m01.tobytes()
                if key not in uniq:
                    uniq[key] = len(tiles)
                    tiles.append(m01)
                row.append((uniq[key], lo))
        plan.append(row)
    if not tiles:  # keep the DRAM tensor non-empty
        tiles.append(np.zeros((128, TC2), np.float16))
    return plan, np.stack(tiles)


def _rope_tables():
    inv_freq = 1.0 / (THETA ** (np.arange(0, D, 2, dtype=np.float64) / D))
    freqs = np.outer(inv_freq, np.arange(T, dtype=np.float64))  # [64, T]
    cosI = np.repeat(np.cos(freqs), 2, axis=0).astype(np.float16)  # [128, T]
    sinI = np.repeat(np.sin(freqs), 2, axis=0).astype(np.float16)
    # rot = psignT.T @ x : rot[2i] = -x[2i+1], rot[2i+1] = x[2i]
    psignT = np.zeros((D, D), np.float16)
    for i in range(D // 2):
        psignT[2 * i + 1, 2 * i] = -1.0
        psignT[2 * i, 2 * i + 1] = 1.0
    return cosI, sinI, psignT


def _phase12(nc, tc, qkv_tensors, xTt, cos_sb, sin_sb, late_loads,
             plan, n_bias, bias_d, ones_sb, a2a_in, a2a_out):
    """Projections with head-0 attention chunks interleaved: chunk
    (h=0, b, ci) only needs k/q/v through t-chunk 4*b+ci, so it runs right
    after that section and the first all-to-all fires at projection end
    instead of mid-attention. PSUM is shared by tag: proj/sc (4 bufs),
    rot/outT (2), vps/r (2) = exactly 8 banks."""
    qT, kT, vt, wq_h, wk_h, wv_sb, psg_sb = qkv_tensors
    with tc.tile_pool(name="xt", bufs=2) as xp, \
         tc.tile_pool(name="p1t", bufs=1) as p1, \
         tc.tile_pool(name="p2t", bufs=1) as p2, \
         tc.tile_pool(name="ps12", bufs=1, space="PSUM") as pp:
        mask_sb = p2.tile([128, n_bias, TC2], F16)
        LA = 2   # sc-matmul lookahead so the PE never waits on ACT's exp

        def emit_chunk(h, b, ci):
            gci = b * CI + ci      # global chunk == dest rank
            live = [(jt, plan[ci][jt]) for jt in range(JT)
                    if plan[ci][jt] is not None]
            if not live:
                z = p2.tile([128, TC2], F16, name="z", tag="ot", bufs=3)
                nc.vector.memset(z[:], 0.0)
                nc.sync.dma_start(a2a_in[h][gci, :, :], z[:])
                return
            # column-support pruning relies on the first live tile
            # starting the full accumulation region
            full_width = live[0][1][1] != 0
            outp = pp.tile([D, TC2], F32, name="outp", tag="mid", bufs=2)
            # ones_sb is [128, 128] all-ones: every partition of rp
            # gets the denominator -- the broadcast is free on PE
            rp = pp.tile([128, TC2], F32, name="rp", tag="vr", bufs=2)
            i0 = b * T + ci * TC2
            nlive = len(live)
            pend = [None] * nlive
            for step in range(nlive + LA):
                if step < nlive:
                    jt, (code, lo) = live[step]
                    if full_width:
                        lo = 0
                    sc = pp.tile([128, TC2], F32, name="sc", tag="big",
                                 bufs=4)
                    nc.tensor.matmul(
                        sc[:, lo:],
                        kT[h][:, b * T + jt * 128:b * T + (jt + 1) * 128],
                        qT[h][:, i0 + lo:i0 + TC2], start=True, stop=True)
                    pt = p2.tile([128, TC2], F16, name="pt", tag="pt",
                                 bufs=4)
                    nc.scalar.activation(
                        pt[:, lo:], sc[:, lo:],
                        mybir.ActivationFunctionType.Exp,
                        bias=0.0, scale=float(SCALE))
                    if code >= 0:
                        ptm = p2.tile([128, TC2], F16, name="ptm",
                                      tag="ptm", bufs=3)
                        nc.vector.tensor_mul(ptm[:, lo:], pt[:, lo:],
                                             mask_sb[:, code, lo:])
                        pt = ptm
                    pend[step] = (pt, lo)
                j = step - LA
                if 0 <= j < nlive:
                    pt_j, lo_j = pend[j]
                    pend[j] = None
                    jv = (b * T) // 128 + live[j][0]
                    nc.tensor.matmul(
                        outp[:, lo_j:],
                        vt[jv][:, h * D:(h + 1) * D], pt_j[:, lo_j:],
                        start=(j == 0), stop=(j == nlive - 1))
                    nc.tensor.matmul(
                        rp[:, lo_j:], ones_sb[:], pt_j[:, lo_j:],
                        start=(j == 0), stop=(j == nlive - 1))
            ri = p2.tile([128, TC2], F32, name="ri", tag="ri", bufs=2)
            nc.vector.reciprocal(ri[:], rp[:])
            ot = p2.tile([128, TC2], F16, name="ot", tag="ot", bufs=3)
            nc.vector.tensor_mul(ot[:], outp[:], ri[:])
            nc.sync.dma_start(a2a_in[h][gci, :, :], ot[:])

        for tcn in range(NTC1):
            ts = tcn * TC1           # position in flattened BT
            tp = ts % T              # rope position (restarts per batch)
            xt = xp.tile([128, KT, TC1], F16, tag="xt")
            nparts = 8 if tcn == 0 else 2
            step = KT // nparts
            for q_ in range(nparts):
                nc.sync.dma_start(xt[:, q_ * step:(q_ + 1) * step, :],
                                  xTt[tcn, :, q_ * step:(q_ + 1) * step, :])
            if tcn == 0:
                late_loads()     # weight DMAs behind the critical first loads
                nc.sync.dma_start(mask_sb[:],
                                  bias_d.rearrange("u p m -> p u m"))
            for dst, w_h in ((qT, wq_h), (kT, wk_h)):
                for h in range(HPC):
                    ps = pp.tile([D, TC1], F32, name="ps", tag="big", bufs=4)
                    for cc in range(KT):
                        nc.tensor.matmul(
                            ps[:], w_h[h][:, cc, :], xt[:, cc, :],
                            start=(cc == 0), stop=(cc == KT - 1))
                    praw = p1.tile([D, TC1], F16, tag="praw", bufs=3)
                    nc.scalar.copy(praw[:], ps[:])
                    rot = pp.tile([D, TC1], F32, name="rot", tag="mid",
                                  bufs=2)
                    nc.tensor.matmul(rot[:], psg_sb[:], praw[:],
                                     start=True, stop=True)
                    rotc = p1.tile([D, TC1], F16, tag="rotc", bufs=2)
                    nc.scalar.copy(rotc[:], rot[:])
                    t1 = p1.tile([D, TC1], F16, tag="t1", bufs=2)
                    nc.vector.tensor_mul(t1[:], praw[:], cos_sb[:, tp:tp + TC1])
                    t2 = p1.tile([D, TC1], F16, tag="t2", bufs=2)
                    nc.gpsimd.tensor_mul(t2[:], rotc[:], sin_sb[:, tp:tp + TC1])
                    nc.vector.tensor_add(dst[h][:, ts:ts + TC1], t1[:], t2[:])
            # v projection: out [t, d] per 128-row t-tile
            for tt in range(TC1 // 128):
                jt = ts // 128 + tt
                ps = pp.tile([128, TC2], F32, name="vps", tag="vr", bufs=2)
                for cc in range(KT):
                    nc.tensor.matmul(
                        ps[:, 0:HPC * D], xt[:, cc, tt * 128:(tt + 1) * 128],
                        wv_sb[:, cc, :],
                        start=(cc == 0), stop=(cc == KT - 1))
                nc.scalar.copy(vt[jt][:], ps[:, 0:HPC * D])
            # head-0 attention for the chunk this section just completed
            emit_chunk(0, tcn // CI, tcn % CI)
        nc.gpsimd.collective_compute(
            "AllToAll", mybir.AluOpType.bypass,
            replica_groups=[list(range(N_CORES))],
            ins=[a2a_in[0].opt()], outs=[a2a_out[0].opt()])
        # head-1 attention runs after the projections, overlapping the
        # first all-to-all's latency
        for b in range(B):
            for ci in range(CI):
                emit_chunk(1, b, ci)
        nc.gpsimd.collective_compute(
            "AllToAll", mybir.AluOpType.bypass,
            replica_groups=[list(range(N_CORES))],
            ins=[a2a_in[1].opt()], outs=[a2a_out[1].opt()])


def _phase3(nc, tc, wop, wot, a2a_out, y):
    # contraction block g = head s*HPC+k lives in a2a_out[k][s]; group by
    # parity k so the k=1 group alone waits on the last AllToAll
    with tc.tile_pool(name="ao", bufs=1) as aop, \
         tc.tile_pool(name="ps3", bufs=1, space="PSUM") as pp:
        ao = {}
        for k in range(HPC):
            for s in range(N_CORES):
                g = s * HPC + k
                t_ = aop.tile([128, TC2], F16, name=f"ao{g}")
                nc.sync.dma_start(t_[:], a2a_out[k][s, :, :])
                ao[g] = t_
        wha = {}
        for cj in range(C // TC2):
            wha[cj] = wop.tile([128, KT, TC2], F16, name=f"wo{cj}")
            for q_ in range(4):
                nc.sync.dma_start(wha[cj][:, 4 * q_:4 * (q_ + 1), :],
                                  wot[cj, :, 4 * q_:4 * (q_ + 1), :])
        g0 = [g for g in range(KT) if g % HPC == 0]
        g1 = [g for g in range(KT) if g % HPC != 0]
        # stage A: y_g0 for ALL output tiles, gated only on the first
        # all-to-all -- fills the entire second-collective window with PE work
        ysb0 = {}
        for cj in range(C // TC2):
            for tt in range(TSL // 128):
                ypa = pp.tile([128, TC2], F32, name="ypa",
                              tag=f"ypa{(cj * 4 + tt) % 4}", bufs=1)
                for n_, g in enumerate(g0):
                    nc.tensor.matmul(
                        ypa[:], ao[g][:, tt * 128:(tt + 1) * 128],
                        wha[cj][:, g, :], start=(n_ == 0),
                        stop=(n_ == len(g0) - 1))
                ysb0[cj, tt] = wop.tile([128, TC2], F32,
                                        name=f"ys0_{cj}_{tt}")
                if tt % 2 == 0:
                    nc.vector.tensor_copy(ysb0[cj, tt][:], ypa[:])
                else:
                    nc.scalar.copy(ysb0[cj, tt][:], ypa[:])
        # stage B: y_g1 (needs the second all-to-all) + add + store
        for cj in range(C // TC2):
            for tt in range(TSL // 128):
                ypb = pp.tile([128, TC2], F32, name="ypb",
                              tag=f"ypb{(cj * 4 + tt) % 4}", bufs=1)
                for n_, g in enumerate(g1):
                    nc.tensor.matmul(
                        ypb[:], ao[g][:, tt * 128:(tt + 1) * 128],
                        wha[cj][:, g, :], start=(n_ == 0),
                        stop=(n_ == len(g1) - 1))
                ysb = wop.tile([128, TC2], F32, tag="ysb", bufs=4)
                nc.vector.tensor_add(ysb[:], ypb[:], ysb0[cj, tt][:])
                for q_ in range(2):
                    nc.sync.dma_start(
                        y[tt * 128:(tt + 1) * 128,
                          cj * TC2 + q_ * (TC2 // 2):
                          cj * TC2 + (q_ + 1) * (TC2 // 2)],
                        ysb[:, q_ * (TC2 // 2):(q_ + 1) * (TC2 // 2)])


def _build(plan, n_bias):
    nc = bacc.Bacc("TRN2", num_devices=N_CORES)

    # host-pre-tiled inputs: contiguous per-partition runs for fat DMA lines
    xTt = nc.dram_tensor("xTt", [NTC1, 128, KT, TC1], F16, kind="ExternalInput")
    wqt = nc.dram_tensor("wqt", [HPC, 128, KT, D], F16, kind="ExternalInput")
    wkt = nc.dram_tensor("wkt", [HPC, 128, KT, D], F16, kind="ExternalInput")
    wvt = nc.dram_tensor("wvt", [128, KT, HPC * D], F16, kind="ExternalInput")
    wot = nc.dram_tensor("wot", [C // TC2, 128, KT, TC2], F16,
                         kind="ExternalInput")
    cos_d = nc.dram_tensor("cos", [D, T], F16, kind="ExternalInput")
    sin_d = nc.dram_tensor("sin", [D, T], F16, kind="ExternalInput")
    psg_d = nc.dram_tensor("psg", [D, D], F16, kind="ExternalInput")
    ones_d = nc.dram_tensor("ones", [128, 128], F16, kind="ExternalInput")
    bias_d = nc.dram_tensor("bias", [n_bias, 128, TC2], F16, kind="ExternalInput")
    y = nc.dram_tensor("y", [TSL, C], F32, kind="ExternalOutput")

    with tile.TileContext(nc) as tc:
        with tc.tile_pool(name="const", bufs=1) as cpool, \
             tc.tile_pool(name="dram", bufs=1, space="DRAM") as dram:

            a2a_in = [dram.tile([N_CORES, D, TC2], F16, name=f"a2ai{h}")
                      for h in range(HPC)]
            a2a_out = [dram.tile([N_CORES, D, TC2], F16, name=f"a2ao{h}")
                       for h in range(HPC)]

            with tc.tile_pool(name="qkv", bufs=1) as qkv:
                qT = [qkv.tile([D, BT], F16, name=f"qT{h}") for h in range(HPC)]
                kT = [qkv.tile([D, BT], F16, name=f"kT{h}") for h in range(HPC)]
                vt = [qkv.tile([128, HPC * D], F16, name=f"v{j}")
                      for j in range(BT // 128)]

                with tc.tile_pool(name="wp", bufs=1) as wp:
                    # only the q-projection weights go ahead of the first x
                    # tile; everything else is issued via late_loads below
                    wq_h = []
                    for h in range(HPC):
                        w_ = wp.tile([128, KT, D], F16, name=f"wqh{h}")
                        nc.sync.dma_start(w_[:], wqt[h])
                        wq_h.append(w_)
                    wk_h = []
                    cos_sb = wp.tile([D, T], F16)
                    sin_sb = wp.tile([D, T], F16)
                    psg_sb = cpool.tile([D, D], F16)
                    ones_sb = cpool.tile([128, 128], F16)
                    wv_sb = wp.tile([128, KT, HPC * D], F16)

                    def late_loads():
                        nc.sync.dma_start(cos_sb[:], cos_d[:])
                        nc.sync.dma_start(sin_sb[:], sin_d[:])
                        for h in range(HPC):
                            w_ = wp.tile([128, KT, D], F16, name=f"wkh{h}")
                            nc.sync.dma_start(w_[:, 0:KT // 2, :],
                                              wkt[h, :, 0:KT // 2, :])
                            nc.sync.dma_start(w_[:, KT // 2:KT, :],
                                              wkt[h, :, KT // 2:KT, :])
                            wk_h.append(w_)
                        nc.sync.dma_start(psg_sb[:], psg_d[:])
                        nc.sync.dma_start(ones_sb[:], ones_d[:])
                        nc.sync.dma_start(wv_sb[:, 0:KT // 2, :],
                                          wvt[:, 0:KT // 2, :])
                        nc.sync.dma_start(wv_sb[:, KT // 2:KT, :],
                                          wvt[:, KT // 2:KT, :])
                        # preload the ACT Exp table so phase 2's first exp
                        # doesn't pay the table switch
                        warm = cpool.tile([128, 1], F32)
                        nc.scalar.activation(warm[:], ones_sb[:, 0:1],
                                             mybir.ActivationFunctionType.Exp,
                                             bias=0.0, scale=1.0)

                    _phase12(nc, tc, (qT, kT, vt, wq_h, wk_h, wv_sb, psg_sb),
                             xTt, cos_sb, sin_sb, late_loads,
                             plan, n_bias, bias_d, ones_sb, a2a_in, a2a_out)

                # wo pool opens as soon as the phase-1 weights are freed so
                # the wo loads stream under the head-1 attention compute
                with tc.tile_pool(name="wo", bufs=1) as wop:
                    _phase3(nc, tc, wop, wot, a2a_out, y)

    nc.finalize()
    return nc


_cache = {}


def _get_kernel(mask2d):
    key = mask2d.tobytes()
    if key not in _cache:
        plan, bias_tiles = _mask_plan(mask2d)
        nc = _build(plan, bias_tiles.shape[0])
        _cache[key] = (nc, bias_tiles)
    return _cache[key]


def kernel(x, mask, wq, wk, wv, wo, _trace=False):
    x = np.asarray(x)
    mask2d = np.asarray(mask).reshape(T, T).astype(bool)
    nc, bias_tiles = _get_kernel(mask2d)

    cosI, sinI, psignT = _rope_tables()
    # [C, BT] -> [NTC1, 128, KT, TC1]: partition p, chunk n <-> row n*128+p
    xT = np.ascontiguousarray(x.reshape(BT, C).T.astype(np.float16))
    xTt = np.ascontiguousarray(
        xT.reshape(KT, 128, NTC1, TC1).transpose(2, 1, 0, 3))

    def pack_w(w):  # [C, HPC*D] -> [HPC, 128, KT, D]
        w16 = np.asarray(w).astype(np.float16)
        return np.ascontiguousarray(
            w16.reshape(KT, 128, HPC, D).transpose(2, 1, 0, 3))

    wo16 = np.asarray(wo).astype(np.float16)   # [N_HEADS*D, C]
    wot = np.ascontiguousarray(
        wo16.reshape(KT, 128, C // TC2, TC2).transpose(2, 1, 0, 3))

    common = {
        "cos": cosI, "sin": sinI, "psg": psignT,
        "ones": np.ones((128, 128), np.float16),
        "bias": bias_tiles, "wot": wot, "xTt": xTt,
    }
    in_maps = []
    for c in range(N_CORES):
        sl = slice(c * HPC * D, (c + 1) * HPC * D)
        wv16 = np.asarray(wv)[:, sl].astype(np.float16)
        in_maps.append({
            "wqt": pack_w(np.asarray(wq)[:, sl]),
            "wkt": pack_w(np.asarray(wk)[:, sl]),
            "wvt": np.ascontiguousarray(
                wv16.reshape(KT, 128, HPC * D).transpose(1, 0, 2)),
            **common,
        })

    r = run_bass_kernel_spmd(nc, in_maps, core_ids=list(range(N_CORES)),
                             trace=_trace)
    out = np.empty((BT, C), np.float32)
    for c in range(N_CORES):
        out[c * TSL:(c + 1) * TSL, :] = r.results[c]["y"]
    if _trace:
        kernel.last_results = r
    return out.reshape(B, T, C)


# revision 46
# speedup vs baseline: 1.0490x; 1.0490x over previous
"""Multi-head causal attention (RoPE) forward on 8 Trainium2 NeuronCores.

Sharding: tensor-parallel over heads -- 8 cores x 2 heads, each core handling
both batch elements (the flattened (B*T) = 4096 "time" axis).

v2: all matmul operands fp16 (PSUM accumulation stays fp32; measured end-to-end
rel err ~1e-3, gate is 2e-2), host-retiled contiguous DMA layouts, TC1=512,
phase-2 bias-adds and broadcasts on the idle GpSimd engine, reciprocal taken
after the partition-broadcast (128-partition op, not 1-partition), per-head
AllToAll in fp16 with the output projection split into head-parity halves so
the last a2a overlaps the first half's matmuls.

Per core:
  phase 1: qT/kT [d, B*T] and v [B*T, d] projections from host-pre-tiled
           xT, RoPE applied via a +-1 pair-swap permutation matmul on PE plus
           elementwise combine with host-precomputed interleaved cos/sin.
  phase 2: per (head, batch), scores^T [j, i] = kT^T @ qT, exp on ScalarE (no
           max pass -- the score distribution is bounded and softmax is
           shift-invariant), mask as additive bias on partially-masked tiles
           only, fully-masked tiles skipped; out^T [d, i] and the softmax
           denominators (ones-matmul) accumulate on PE; normalized on evac.
           Each head's attention output leaves via its own 8-rank AllToAll
           (head-split -> t-split) so comm overlaps the next head's compute.
  phase 3: y[t-slice, :] = outT_full^T @ wo, accumulated in two head-parity
           groups so only the first group waits on the final AllToAll.
Host assembles the 8 t-slices into the full (B, T, C) output.
"""

import os
import sys

import numpy as np

for _p in ("/opt/trn_rl_repo", "/root/.axon_site/_ro/trn_rl_repo"):
    if os.path.isdir(_p) and _p not in sys.path:
        sys.path.append(_p)

import concourse.bacc as bacc
import concourse.tile as tile
from concourse import mybir
from concourse.bass_utils import run_bass_kernel_spmd

B, T, C = 2, 2048, 2048
N_HEADS, D = 16, 128
THETA = 10000.0
N_CORES = 8
HPC = N_HEADS // N_CORES     # heads per core
BT = B * T                   # flattened time axis
TSL = BT // N_CORES          # per-core output slice after the all-to-all
KT = C // 128                # contraction chunks
TC1 = 512                    # phase-1 t-chunk (moving free dim)
NTC1 = BT // TC1
TC2 = 512                    # phase-2/3 chunk
CI = T // TC2                # i-chunks per (head, batch)
JT = T // 128                # j-tiles per (head, batch)
SCALE = 1.0 / np.sqrt(D)
MASKED_BIAS = -1.0e6         # pre-scale units; exp(SCALE*(s+bias)) == 0

F16 = mybir.dt.float16
F32 = mybir.dt.float32


def _mask_plan(mask2d):
    """Per (ci, jt): None=skip (all masked), (-1, 0)=free (none masked),
    (idx, lo) = partial tile whose columns [lo, TC2) have at least one valid
    row; idx is a 0/1 multiplier tile applied to exp(scores) on that column
    range. scoresT tile (jt, ci) holds mask2d[i, j] transposed:
    m01[j_loc, i_loc] <- mask2d[TC2*ci+i, 128*jt+j].
    """
    uniq = {}
    tiles = []
    plan = []
    for ci in range(CI):
        row = []
        for jt in range(JT):
            blk = mask2d[TC2 * ci:TC2 * (ci + 1), 128 * jt:128 * (jt + 1)]
            if blk.all():
                row.append((-1, 0))
            elif not blk.any():
                row.append(None)
            else:
                m01 = blk.T.astype(np.float16)   # [128 j, TC2 i]
                lo = int(np.argmax(m01.any(axis=0)))
                key = m01.tobytes()
                if key not in uniq:
                    uniq[key] = len(tiles)
                    tiles.append(m01)
                row.append((uniq[key], lo))
        plan.append(row)
    if not tiles:  # keep the DRAM tensor non-empty
        tiles.append(np.zeros((128, TC2), np.float16))
    return plan, np.stack(tiles)


def _rope_tables():
    inv_freq = 1.0 / (THETA ** (np.arange(0, D, 2, dtype=np.float64) / D))
    freqs = np.outer(inv_freq, np.arange(T, dtype=np.float64))  # [64, T]
    cosI = np.repeat(np.cos(freqs), 2, axis=0).astype(np.float16)  # [128, T]
    sinI = np.repeat(np.sin(freqs), 2, axis=0).astype(np.float16)
    # rot = psignT.T @ x : rot[2i] = -x[2i+1], rot[2i+1] = x[2i]
    psignT = np.zeros((D, D), np.float16)
    for i in range(D // 2):
        psignT[2 * i + 1, 2 * i] = -1.0
        psignT[2 * i, 2 * i + 1] = 1.0
    return cosI, sinI, psignT


def _phase12(nc, tc, qkv_tensors, xTt, cos_sb, sin_sb, late_loads,
             plan, n_bias, bias_d, ones_sb, a2a_in, a2a_out, qkv, wot, wha):
    """Projections with head-0 attention chunks interleaved: chunk
    (h=0, b, ci) only needs k/q/v through t-chunk 4*b+ci, so it runs right
    after that section and the first all-to-all fires at projection end
    instead of mid-attention. PSUM is shared by tag: proj/sc (4 bufs),
    rot/outT (2), vps/r (2) = exactly 8 banks."""
    qT, kT, vt, wq_h, wk_h, wv_sb, psg_sb = qkv_tensors
    with tc.tile_pool(name="xt", bufs=2) as xp, \
         tc.tile_pool(name="p1t", bufs=1) as p1, \
         tc.tile_pool(name="p2t", bufs=1) as p2, \
         tc.tile_pool(name="ps12", bufs=1, space="PSUM") as pp:
        mask_sb = p2.tile([128, n_bias, TC2], F16)
        LA = 2   # sc-matmul lookahead so the PE never waits on ACT's exp

        def emit_chunk(h, b, ci):
            gci = b * CI + ci      # global chunk == dest rank
            live = [(jt, plan[ci][jt]) for jt in range(JT)
                    if plan[ci][jt] is not None]
            if not live:
                z = p2.tile([128, TC2], F16, name="z", tag="ot", bufs=3)
                nc.vector.memset(z[:], 0.0)
                nc.sync.dma_start(a2a_in[h][gci, :, :], z[:])
                return
            # column-support pruning relies on the first live tile
            # starting the full accumulation region
            full_width = live[0][1][1] != 0
            outp = pp.tile([D, TC2], F32, name="outp", tag="mid", bufs=2)
            # ones_sb is [128, 128] all-ones: every partition of rp
            # gets the denominator -- the broadcast is free on PE
            rp = pp.tile([128, TC2], F32, name="rp", tag="vr", bufs=2)
            i0 = b * T + ci * TC2
            nlive = len(live)
            pend = [None] * nlive
            for step in range(nlive + LA):
                if step < nlive:
                    jt, (code, lo) = live[step]
                    if full_width:
                        lo = 0
                    sc = pp.tile([128, TC2], F32, name="sc", tag="big",
                                 bufs=4)
                    nc.tensor.matmul(
                        sc[:, lo:],
                        kT[h][:, b * T + jt * 128:b * T + (jt + 1) * 128],
                        qT[h][:, i0 + lo:i0 + TC2], start=True, stop=True)
                    pt = p2.tile([128, TC2], F16, name="pt", tag="pt",
                                 bufs=4)
                    nc.scalar.activation(
                        pt[:, lo:], sc[:, lo:],
                        mybir.ActivationFunctionType.Exp,
                        bias=0.0, scale=float(SCALE))
                    if code >= 0:
                        ptm = p2.tile([128, TC2], F16, name="ptm",
                                      tag="ptm", bufs=3)
                        nc.vector.tensor_mul(ptm[:, lo:], pt[:, lo:],
                                             mask_sb[:, code, lo:])
                        pt = ptm
                    pend[step] = (pt, lo)
                j = step - LA
                if 0 <= j < nlive:
                    pt_j, lo_j = pend[j]
                    pend[j] = None
                    jv = (b * T) // 128 + live[j][0]
                    nc.tensor.matmul(
                        outp[:, lo_j:],
                        vt[jv][:, h * D:(h + 1) * D], pt_j[:, lo_j:],
                        start=(j == 0), stop=(j == nlive - 1))
                    nc.tensor.matmul(
                        rp[:, lo_j:], ones_sb[:], pt_j[:, lo_j:],
                        start=(j == 0), stop=(j == nlive - 1))
            ri = p2.tile([128, TC2], F32, name="ri", tag="ri", bufs=2)
            # ~18 correct bits, 5x faster than reciprocal(); r >= exp(s_ii)
            # here so the undefined edge cases cannot occur
            nc.vector.reciprocal_approx_fast(ri[:], rp[:])
            ot = p2.tile([128, TC2], F16, name="ot", tag="ot", bufs=3)
            nc.vector.tensor_mul(ot[:], outp[:], ri[:])
            nc.sync.dma_start(a2a_in[h][gci, :, :], ot[:])

        for tcn in range(NTC1):
            ts = tcn * TC1           # position in flattened BT
            tp = ts % T              # rope position (restarts per batch)
            xt = xp.tile([128, KT, TC1], F16, tag="xt")
            nparts = 8 if tcn == 0 else 2
            step = KT // nparts
            for q_ in range(nparts):
                nc.sync.dma_start(xt[:, q_ * step:(q_ + 1) * step, :],
                                  xTt[tcn, :, q_ * step:(q_ + 1) * step, :])
            if tcn == 0:
                late_loads()     # weight DMAs behind the critical first loads
                nc.sync.dma_start(mask_sb[:],
                                  bias_d.rearrange("u p m -> p u m"))
            if tcn == NTC1 // 2:
                # prefetch half the output-projection weights in the DMA-idle
                # mid-projection window, so the remaining loads after the
                # weight pool frees don't contend with the ot evacuations
                # that gate the last all-to-all
                for cj in range(2):
                    wha[cj] = qkv.tile([128, KT, TC2], F16, name=f"wo{cj}")
                    for q_ in range(4):
                        nc.sync.dma_start(
                            wha[cj][:, 4 * q_:4 * (q_ + 1), :],
                            wot[cj, :, 4 * q_:4 * (q_ + 1), :])
            for dst, w_h in ((qT, wq_h), (kT, wk_h)):
                for h in range(HPC):
                    ps = pp.tile([D, TC1], F32, name="ps", tag="big", bufs=4)
                    for cc in range(KT):
                        nc.tensor.matmul(
                            ps[:], w_h[h][:, cc, :], xt[:, cc, :],
                            start=(cc == 0), stop=(cc == KT - 1))
                    praw = p1.tile([D, TC1], F16, tag="praw", bufs=3)
                    nc.scalar.copy(praw[:], ps[:])
                    rot = pp.tile([D, TC1], F32, name="rot", tag="mid",
                                  bufs=2)
                    nc.tensor.matmul(rot[:], psg_sb[:], praw[:],
                                     start=True, stop=True)
                    rotc = p1.tile([D, TC1], F16, tag="rotc", bufs=2)
                    nc.scalar.copy(rotc[:], rot[:])
                    t1 = p1.tile([D, TC1], F16, tag="t1", bufs=2)
                    nc.vector.tensor_mul(t1[:], praw[:], cos_sb[:, tp:tp + TC1])
                    t2 = p1.tile([D, TC1], F16, tag="t2", bufs=2)
                    nc.gpsimd.tensor_mul(t2[:], rotc[:], sin_sb[:, tp:tp + TC1])
                    nc.vector.tensor_add(dst[h][:, ts:ts + TC1], t1[:], t2[:])
            # v projection: out [t, d] per 128-row t-tile
            for tt in range(TC1 // 128):
                jt = ts // 128 + tt
                ps = pp.tile([128, TC2], F32, name="vps", tag="vr", bufs=2)
                for cc in range(KT):
                    nc.tensor.matmul(
                        ps[:, 0:HPC * D], xt[:, cc, tt * 128:(tt + 1) * 128],
                        wv_sb[:, cc, :],
                        start=(cc == 0), stop=(cc == KT - 1))
                nc.scalar.copy(vt[jt][:], ps[:, 0:HPC * D])
            # head-0 attention for the chunk this section just completed
            emit_chunk(0, tcn // CI, tcn % CI)
        nc.gpsimd.collective_compute(
            "AllToAll", mybir.AluOpType.bypass,
            replica_groups=[list(range(N_CORES))],
            ins=[a2a_in[0].opt()], outs=[a2a_out[0].opt()])
        # head-1 attention runs after the projections, overlapping the
        # first all-to-all's latency
        for b in range(B):
            for ci in range(CI):
                emit_chunk(1, b, ci)
        nc.gpsimd.collective_compute(
            "AllToAll", mybir.AluOpType.bypass,
            replica_groups=[list(range(N_CORES))],
            ins=[a2a_in[1].opt()], outs=[a2a_out[1].opt()])


def _phase3(nc, tc, wop, wot, a2a_out, y, wha):
    # contraction block g = head s*HPC+k lives in a2a_out[k][s]; group by
    # parity k so the k=1 group alone waits on the last AllToAll
    with tc.tile_pool(name="ao", bufs=1) as aop, \
         tc.tile_pool(name="ps3", bufs=1, space="PSUM") as pp:
        ao = {}
        for k in range(HPC):
            for s in range(N_CORES):
                g = s * HPC + k
                t_ = aop.tile([128, TC2], F16, name=f"ao{g}")
                nc.sync.dma_start(t_[:], a2a_out[k][s, :, :])
                ao[g] = t_
        for cj in range(2, C // TC2):
            wha[cj] = wop.tile([128, KT, TC2], F16, name=f"wo{cj}")
            for q_ in range(4):
                nc.sync.dma_start(wha[cj][:, 4 * q_:4 * (q_ + 1), :],
                                  wot[cj, :, 4 * q_:4 * (q_ + 1), :])
        g0 = [g for g in range(KT) if g % HPC == 0]
        g1 = [g for g in range(KT) if g % HPC != 0]
        # stage A: y_g0 for ALL output tiles, gated only on the first
        # all-to-all -- fills the entire second-collective window with PE work
        ysb0 = {}
        for cj in range(C // TC2):
            for tt in range(TSL // 128):
                ypa = pp.tile([128, TC2], F32, name="ypa",
                              tag=f"ypa{(cj * 4 + tt) % 4}", bufs=1)
                for n_, g in enumerate(g0):
                    nc.tensor.matmul(
                        ypa[:], ao[g][:, tt * 128:(tt + 1) * 128],
                        wha[cj][:, g, :], start=(n_ == 0),
                        stop=(n_ == len(g0) - 1))
                ysb0[cj, tt] = wop.tile([128, TC2], F32,
                                        name=f"ys0_{cj}_{tt}")
                if tt % 2 == 0:
                    nc.vector.tensor_copy(ysb0[cj, tt][:], ypa[:])
                else:
                    nc.scalar.copy(ysb0[cj, tt][:], ypa[:])
        # stage B: y_g1 (needs the second all-to-all) + add + store
        for cj in range(C // TC2):
            for tt in range(TSL // 128):
                ypb = pp.tile([128, TC2], F32, name="ypb",
                              tag=f"ypb{(cj * 4 + tt) % 4}", bufs=1)
                for n_, g in enumerate(g1):
                    nc.tensor.matmul(
                        ypb[:], ao[g][:, tt * 128:(tt + 1) * 128],
                        wha[cj][:, g, :], start=(n_ == 0),
                        stop=(n_ == len(g1) - 1))
                ysb = wop.tile([128, TC2], F32, tag="ysb", bufs=4)
                nc.vector.tensor_add(ysb[:], ypb[:], ysb0[cj, tt][:])
                for q_ in range(2):
                    nc.sync.dma_start(
                        y[tt * 128:(tt + 1) * 128,
                          cj * TC2 + q_ * (TC2 // 2):
                          cj * TC2 + (q_ + 1) * (TC2 // 2)],
                        ysb[:, q_ * (TC2 // 2):(q_ + 1) * (TC2 // 2)])


def _build(plan, n_bias):
    nc = bacc.Bacc("TRN2", num_devices=N_CORES)

    # host-pre-tiled inputs: contiguous per-partition runs for fat DMA lines
    xTt = nc.dram_tensor("xTt", [NTC1, 128, KT, TC1], F16, kind="ExternalInput")
    wqt = nc.dram_tensor("wqt", [HPC, 128, KT, D], F16, kind="ExternalInput")
    wkt = nc.dram_tensor("wkt", [HPC, 128, KT, D], F16, kind="ExternalInput")
    wvt = nc.dram_tensor("wvt", [128, KT, HPC * D], F16, kind="ExternalInput")
    wot = nc.dram_tensor("wot", [C // TC2, 128, KT, TC2], F16,
                         kind="ExternalInput")
    cos_d = nc.dram_tensor("cos", [D, T], F16, kind="ExternalInput")
    sin_d = nc.dram_tensor("sin", [D, T], F16, kind="ExternalInput")
    psg_d = nc.dram_tensor("psg", [D, D], F16, kind="ExternalInput")
    ones_d = nc.dram_tensor("ones", [128, 128], F16, kind="ExternalInput")
    bias_d = nc.dram_tensor("bias", [n_bias, 128, TC2], F16, kind="ExternalInput")
    y = nc.dram_tensor("y", [TSL, C], F32, kind="ExternalOutput")

    with tile.TileContext(nc) as tc:
        with tc.tile_pool(name="const", bufs=1) as cpool, \
             tc.tile_pool(name="dram", bufs=1, space="DRAM") as dram:

            a2a_in = [dram.tile([N_CORES, D, TC2], F16, name=f"a2ai{h}")
                      for h in range(HPC)]
            a2a_out = [dram.tile([N_CORES, D, TC2], F16, name=f"a2ao{h}")
                       for h in range(HPC)]

            with tc.tile_pool(name="qkv", bufs=1) as qkv:
                qT = [qkv.tile([D, BT], F16, name=f"qT{h}") for h in range(HPC)]
                kT = [qkv.tile([D, BT], F16, name=f"kT{h}") for h in range(HPC)]
                vt = [qkv.tile([128, HPC * D], F16, name=f"v{j}")
                      for j in range(BT // 128)]
                wha = {}

                with tc.tile_pool(name="wp", bufs=1) as wp:
                    # only the q-projection weights go ahead of the first x
                    # tile; everything else is issued via late_loads below
                    wq_h = []
                    for h in range(HPC):
                        w_ = wp.tile([128, KT, D], F16, name=f"wqh{h}")
                        nc.sync.dma_start(w_[:], wqt[h])
                        wq_h.append(w_)
                    wk_h = []
                    cos_sb = wp.tile([D, T], F16)
                    sin_sb = wp.tile([D, T], F16)
                    psg_sb = cpool.tile([D, D], F16)
                    ones_sb = cpool.tile([128, 128], F16)
                    wv_sb = wp.tile([128, KT, HPC * D], F16)

                    def late_loads():
                        nc.sync.dma_start(cos_sb[:], cos_d[:])
                        nc.sync.dma_start(sin_sb[:], sin_d[:])
                        for h in range(HPC):
                            w_ = wp.tile([128, KT, D], F16, name=f"wkh{h}")
                            nc.sync.dma_start(w_[:, 0:KT // 2, :],
                                              wkt[h, :, 0:KT // 2, :])
                            nc.sync.dma_start(w_[:, KT // 2:KT, :],
                                              wkt[h, :, KT // 2:KT, :])
                            wk_h.append(w_)
                        nc.sync.dma_start(psg_sb[:], psg_d[:])
                        nc.sync.dma_start(ones_sb[:], ones_d[:])
                        nc.sync.dma_start(wv_sb[:, 0:KT // 2, :],
                                          wvt[:, 0:KT // 2, :])
                        nc.sync.dma_start(wv_sb[:, KT // 2:KT, :],
                                          wvt[:, KT // 2:KT, :])
                        # preload the ACT Exp table so phase 2's first exp
                        # doesn't pay the table switch
                        warm = cpool.tile([128, 1], F32)
                        nc.scalar.activation(warm[:], ones_sb[:, 0:1],
                                             mybir.ActivationFunctionType.Exp,
                                             bias=0.0, scale=1.0)

                    _phase12(nc, tc, (qT, kT, vt, wq_h, wk_h, wv_sb, psg_sb),
                             xTt, cos_sb, sin_sb, late_loads,
                             plan, n_bias, bias_d, ones_sb, a2a_in, a2a_out,
                             qkv, wot, wha)

                # wo pool opens as soon as the phase-1 weights are freed so
                # the remaining wo loads stream under the head-1 attention
                with tc.tile_pool(name="wo", bufs=1) as wop:
                    _phase3(nc, tc, wop, wot, a2a_out, y, wha)

    nc.finalize()
    return nc


_cache = {}


def _get_kernel(mask2d):
    key = mask2d.tobytes()
    if key not in _cache:
        plan, bias_tiles = _mask_plan(mask2d)
        nc = _build(plan, bias_tiles.shape[0])
        _cache[key] = (nc, bias_tiles)
    return _cache[key]


def kernel(x, mask, wq, wk, wv, wo, _trace=False):
    x = np.asarray(x)
    mask2d = np.asarray(mask).reshape(T, T).astype(bool)
    nc, bias_tiles = _get_kernel(mask2d)

    cosI, sinI, psignT = _rope_tables()
    # [C, BT] -> [NTC1, 128, KT, TC1]: partition p, chunk n <-> row n*128+p
    xT = np.ascontiguousarray(x.reshape(BT, C).T.astype(np.float16))
    xTt = np.ascontiguousarray(
        xT.reshape(KT, 128, NTC1, TC1).transpose(2, 1, 0, 3))

    def pack_w(w):  # [C, HPC*D] -> [HPC, 128, KT, D]
        w16 = np.asarray(w).astype(np.float16)
        return np.ascontiguousarray(
            w16.reshape(KT, 128, HPC, D).transpose(2, 1, 0, 3))

    wo16 = np.asarray(wo).astype(np.float16)   # [N_HEADS*D, C]
    wot = np.ascontiguousarray(
        wo16.reshape(KT, 128, C // TC2, TC2).transpose(2, 1, 0, 3))

    common = {
        "cos": cosI, "sin": sinI, "psg": psignT,
        "ones": np.ones((128, 128), np.float16),
        "bias": bias_tiles, "wot": wot, "xTt": xTt,
    }
    in_maps = []
    for c in range(N_CORES):
        sl = slice(c * HPC * D, (c + 1) * HPC * D)
        wv16 = np.asarray(wv)[:, sl].astype(np.float16)
        in_maps.append({
            "wqt": pack_w(np.asarray(wq)[:, sl]),
            "wkt": pack_w(np.asarray(wk)[:, sl]),
            "wvt": np.ascontiguousarray(
                wv16.reshape(KT, 128, HPC * D).transpose(1, 0, 2)),
            **common,
        })

    r = run_bass_kernel_spmd(nc, in_maps, core_ids=list(range(N_CORES)),
                             trace=_trace)
    out = np.empty((BT, C), np.float32)
    for c in range(N_CORES):
        out[c * TSL:(c + 1) * TSL, :] = r.results[c]["y"]
    if _trace:
        kernel.last_results = r
    return out.reshape(B, T, C)
